# revision 4
# baseline (speedup 1.0000x reference)
"""ChebNetConv (K=4) Bass kernel for 8 trn2 NeuronCores.

Strategy (1D row partitioning per sharding hint):
  - Nodes sharded across 8 cores (12500 rows each). Each SpMM step computes
    the core's own output rows; full neighbor tables (T1 / T2) are available
    to every core via AllGather.
  - SpMM core: edges grouped by (dest block of 128 rows, src chunk of 25000
    rows), padded to batches of 128.  Per batch: source rows land in an SBUF
    tile G[128e, 128f]; a selector tile S[128e, 128d] (Laplacian value at
    (e, dest-in-block)) is built ON-CHIP by the vector engine from a compact
    (dst, val) stream: S = (iota == dst) * val; PE matmul accumulates
    S.T @ G into the dest block's PSUM accumulator.
  - Step 1 needs no gather at all: x is a host input, so the edge-ordered
    gather x[cols] is precomputed on the host and streamed contiguously.
    Steps 2/3 use gpsimd dma_gather from the all-gathered T1/T2 tables.
  - Chebyshev recurrence (T2 = 2*L@T1 - T0) folded into PSUM eviction.
  - Final linear: per dest block, PE-transpose cheb tiles to [f, n] and
    accumulate 4 matmuls against W slices + bias outer product.
"""

import numpy as np

import concourse.bacc as bacc
import concourse.bass as bass
import concourse.mybir as mybir
import concourse.tile as tile
from concourse import bass_utils
from concourse.bass import ds
from concourse.masks import make_identity

P = 128


class Cfg:
    def __init__(self, n_nodes=100000, f=128, k=4, cores=8, chunk=25000, superb=8,
                 maxb=8):
        assert n_nodes % cores == 0
        self.N = n_nodes
        self.F = f
        self.K = k
        self.CORES = cores
        self.RPC = n_nodes // cores            # rows per core
        self.NBLK = -(-self.RPC // P)          # dest blocks per core
        self.CHUNK = chunk                     # gather-table chunk rows (int16 idx limit)
        assert chunk <= 32767
        self.NCHUNK = -(-n_nodes // chunk)
        self.SUPER = superb                    # dest blocks per super-block
        self.NSUPER = -(-self.NBLK // superb)
        self.MAXB = maxb                       # batches per dma_gather sub-call

    def blocks_of(self, s):
        return range(s * self.SUPER, min(self.NBLK, (s + 1) * self.SUPER))

    def nrows_of(self, b):
        return min(P, self.RPC - b * P)


def preprocess(cfg, rows, cols, vals):
    """Build per-core gather-index, (dst,val), and source-index streams.

    Returns (meta, per_core) where meta has compile-time batch counts
    (identical across cores) and per_core[c] = dict of arrays.
    """
    import ml_dtypes
    rows = np.asarray(rows).astype(np.int64)
    cols = np.asarray(cols).astype(np.int64)
    vals = np.asarray(vals).astype(np.float32)

    core = rows // cfg.RPC
    loc = rows % cfg.RPC
    blk = loc // P
    dst = loc % P
    chk = cols // cfg.CHUNK
    src = (cols % cfg.CHUNK).astype(np.int64)

    counts = np.zeros((cfg.CORES, cfg.NBLK, cfg.NCHUNK), dtype=np.int64)
    np.add.at(counts, (core, blk, chk), 1)
    NB = np.maximum(1, -(-counts.max(axis=0) // P))  # [NBLK, NCHUNK] batches

    slot_start = np.zeros((cfg.NBLK, cfg.NCHUNK), dtype=np.int64)
    call_start = {}          # (s, c) -> padded-edge offset of the gather call
    call_nbatch = {}         # (s, c) -> total batches in call
    off = 0
    for s in range(cfg.NSUPER):
        for c in range(cfg.NCHUNK):
            call_start[(s, c)] = off
            nb = 0
            for b in cfg.blocks_of(s):
                slot_start[b, c] = off
                off += NB[b, c] * P
                nb += NB[b, c]
            call_nbatch[(s, c)] = nb
    tot_pad = off
    NBT = tot_pad // P

    meta = dict(NB=NB, call_start=call_start, call_nbatch=call_nbatch,
                tot_pad=tot_pad, NBT=NBT)

    per_core = []
    for cid in range(cfg.CORES):
        m = core == cid
        key = (blk[m] * cfg.NCHUNK + chk[m])
        order = np.argsort(key, kind="stable")
        kb, kc, ksrc, kdst, kval = (blk[m][order], chk[m][order],
                                    src[m][order], dst[m][order],
                                    vals[m][order])
        # rank within slot
        cnt = counts[cid].reshape(-1)
        slot_flat = kb * cfg.NCHUNK + kc
        starts = np.zeros(cfg.NBLK * cfg.NCHUNK, dtype=np.int64)
        starts[1:] = np.cumsum(cnt)[:-1]
        rank = np.arange(len(kb)) - starts[slot_flat]
        pos = slot_start.reshape(-1)[slot_flat] + rank  # padded global position

        q = pos // P          # batch index
        e = pos % P           # lane within batch

        idx_flat = np.zeros(tot_pad, dtype=np.int16)
        idx_flat[pos] = ksrc.astype(np.int16)

        # compact per-lane (dst, val) stream [128, NBT, 2] f32
        dv = np.zeros((P, NBT, 2), dtype=np.float32)
        dv[e, q, 0] = kdst
        dv[e, q, 1] = kval

        # global source index per lane [128, NBT] (padding -> row 0, val=0)
        src_mat = np.zeros((P, NBT), dtype=np.int64)
        src_mat[e, q] = kc * cfg.CHUNK + ksrc

        # idx DMA layout: per call, [128, 8*nb] with idx j at
        # [16g + j%16, j//16] for replica groups g=0..7
        idx_parts = []
        for s in range(cfg.NSUPER):
            for c in range(cfg.NCHUNK):
                o = call_start[(s, c)]
                nb = call_nbatch[(s, c)]
                iv = idx_flat[o:o + nb * P]            # [nb*128]
                arr = iv.reshape(-1, 16).T             # [16, 8*nb]
                idx_parts.append(np.tile(arr, (8, 1)).reshape(-1))
        per_core.append(dict(
            idx_all=np.concatenate(idx_parts),
            dv_all=dv,
            src_mat=src_mat,
        ))
    return meta, per_core


def len_idx(cfg, meta):
    return meta["tot_pad"] * 8  # 128 parts * 8*nb cols per call of nb*128 idxs


def build(cfg, meta):
    """Build the Bass program. Returns nc."""
    NB = meta["NB"]
    NBT = meta["NBT"]
    f32 = mybir.dt.float32
    bf16 = mybir.dt.bfloat16
    nc = bacc.Bacc("TRN2", target_bir_lowering=False, debug=False,
                   num_devices=cfg.CORES, num_swdge_queues=4)

    x_e = nc.dram_tensor("x_e", [P, NBT, cfg.F], bf16, kind="ExternalInput")
    x_shard = nc.dram_tensor("x_shard", [cfg.RPC, cfg.F], f32,
                             kind="ExternalInput")
    idx_in = nc.dram_tensor("idx_all", [len_idx(cfg, meta)], mybir.dt.int16,
                            kind="ExternalInput")
    dv_in = nc.dram_tensor("dv_all", [P, NBT, 2], f32, kind="ExternalInput")
    iota_in = nc.dram_tensor("iota_rep", [P, P], bf16, kind="ExternalInput")
    w_in = nc.dram_tensor("w_lhsT", [cfg.F, cfg.K * cfg.F], f32,
                          kind="ExternalInput")
    b_in = nc.dram_tensor("b_row", [1, cfg.F], f32, kind="ExternalInput")
    out_shard = nc.dram_tensor("out_shard", [cfg.RPC, cfg.F], f32,
                               kind="ExternalOutput")

    rg = [list(range(cfg.CORES))]

    with tile.TileContext(nc) as tc:
        with tc.tile_pool(name="dram", bufs=1, space="DRAM") as dram:
            t1_shard = dram.tile([cfg.RPC, cfg.F], f32, tag="t1s")
            t2_shard = dram.tile([cfg.RPC, cfg.F], f32, tag="t2s")
            t3_shard = dram.tile([cfg.RPC, cfg.F], f32, tag="t3s")
            t1_tab = dram.tile([cfg.N, cfg.F], bf16, tag="t1t",
                               addr_space="Shared")
            t2_tab = dram.tile([cfg.N, cfg.F], bf16, tag="t2t",
                               addr_space="Shared")
            t1_bsh = dram.tile([cfg.RPC, cfg.F], bf16, tag="t1b")
            t2_bsh = dram.tile([cfg.RPC, cfg.F], bf16, tag="t2b")

            with (
                tc.tile_pool(name="gconst", bufs=1) as gconst,
                tc.tile_pool(name="gpool", bufs=2) as gpool,
                tc.tile_pool(name="spool", bufs=2) as spool,
                tc.tile_pool(name="ipool", bufs=2) as ipool,
                tc.tile_pool(name="dvpool", bufs=2) as dvpool,
                tc.tile_pool(name="psum", bufs=2, space="PSUM") as pspool,
                tc.tile_pool(name="ev", bufs=4) as evpool,
            ):
                iota_t = gconst.tile([P, P], bf16)
                nc.sync.dma_start(iota_t[:], iota_in[:])
                for step in (1, 2, 3):
                    src = {1: None, 2: t1_tab[:], 3: t2_tab[:]}[step]
                    prev = {1: None, 2: x_shard, 3: t1_shard}[step]
                    dst = {1: t1_shard, 2: t2_shard, 3: t3_shard}[step]
                    bdst = {1: t1_bsh, 2: t2_bsh, 3: None}[step]
                    spmm_step(cfg, meta, nc, tc, gpool, spool, ipool, dvpool,
                              pspool, evpool, idx_in, dv_in, iota_t, x_e, src,
                              prev, dst, step, bdst)
                    if step == 1:
                        nc.gpsimd.collective_compute(
                            "AllGather", mybir.AluOpType.bypass,
                            replica_groups=rg, ins=[t1_bsh[:].opt()],
                            outs=[t1_tab[:].opt()])
                    elif step == 2:
                        nc.gpsimd.collective_compute(
                            "AllGather", mybir.AluOpType.bypass,
                            replica_groups=rg, ins=[t2_bsh[:].opt()],
                            outs=[t2_tab[:].opt()])

            with (
                tc.tile_pool(name="fconst", bufs=1) as fconst,
                tc.tile_pool(name="fload", bufs=3) as fload,
                tc.tile_pool(name="ftrans", bufs=3) as ftrans,
                tc.tile_pool(name="fpsum", bufs=2, space="PSUM") as fpsum,
                tc.tile_pool(name="fout", bufs=3) as foutp,
            ):
                ident = fconst.tile([P, P], f32)
                make_identity(nc, ident[:])
                wt = fconst.tile([cfg.F, cfg.K, cfg.F], f32)
                nc.sync.dma_start(wt[:], w_in[:].rearrange(
                    "f (k o) -> f k o", k=cfg.K))
                brow = fconst.tile([1, cfg.F], f32)
                nc.sync.dma_start(brow[:], b_in[:])
                ones = fconst.tile([1, P], f32)
                nc.vector.memset(ones[:], 1.0)

                shards = [x_shard, t1_shard, t2_shard, t3_shard]
                for b in range(cfg.NBLK):
                    nrows = cfg.nrows_of(b)
                    r0 = b * P
                    opsum = fpsum.tile([P, cfg.F], f32, tag="opsum")
                    for k in range(cfg.K):
                        ct = fload.tile([P, cfg.F], f32, tag="cheb")
                        sh = shards[k]
                        nc.sync.dma_start(ct[:nrows, :],
                                          sh[r0:r0 + nrows, :])
                        tp = fpsum.tile([P, P], f32, tag="tpsum")
                        nc.tensor.transpose(tp[:, :nrows], ct[:nrows, :],
                                            ident[:nrows, :nrows])
                        cT = ftrans.tile([cfg.F, P], f32, tag="chebT")
                        nc.vector.tensor_copy(cT[:, :nrows], tp[:, :nrows])
                        nc.tensor.matmul(opsum[:nrows, :], cT[:, :nrows],
                                         wt[:, k, :], start=(k == 0),
                                         stop=False)
                    nc.tensor.matmul(opsum[:nrows, :], ones[:1, :nrows],
                                     brow[:1, :], start=False, stop=True)
                    ot = foutp.tile([P, cfg.F], f32, tag="ot")
                    nc.vector.tensor_copy(ot[:nrows, :], opsum[:nrows, :])
                    nc.scalar.dma_start(out_shard[r0:r0 + nrows, :],
                                        ot[:nrows, :])

    nc.compile()
    return nc


def spmm_step(cfg, meta, nc, tc, gpool, spool, ipool, dvpool, pspool, evpool,
              idx_in, dv_in, iota_t, x_e, src, prev, dst, step, bdst=None):
    NB = meta["NB"]
    f32 = mybir.dt.float32
    bf16 = mybir.dt.bfloat16
    sub = mybir.AluOpType.subtract
    iseq = mybir.AluOpType.is_equal
    mult = mybir.AluOpType.mult
    iofs = 0
    qctr = 0
    for s in range(cfg.NSUPER):
        blocks = list(cfg.blocks_of(s))
        ps = [pspool.tile([P, 4, cfg.F], f32, tag=f"ps{i}", name=f"ps{i}")
              for i in range(-(-len(blocks) // 4))]
        for c in range(cfg.NCHUNK):
            nb = meta["call_nbatch"][(s, c)]
            bofs = meta["call_start"][(s, c)] // P   # batch offset in streams
            w8 = nb * 8
            dvt = dvpool.tile([P, nb, 2], f32, tag="dv")
            nc.sync.dma_start(dvt[:], dv_in[:, bofs:bofs + nb, :])
            g = gpool.tile([P, nb, cfg.F], bf16, tag="G")
            if step == 1:
                nc.sync.dma_start(g[:], x_e[:, bofs:bofs + nb, :])
            else:
                ix = ipool.tile([P, w8], mybir.dt.int16, tag="ix")
                nc.sync.dma_start(
                    ix[:], idx_in[iofs:iofs + P * w8].rearrange(
                        "(p w) -> p w", p=P))
                lo = c * cfg.CHUNK
                hi = min(cfg.N, lo + cfg.CHUNK)
                # split into sub-calls: very large dma_gather calls (~10k
                # descriptors) crash/hang the device
                for b0 in range(0, nb, cfg.MAXB):
                    b1 = min(nb, b0 + cfg.MAXB)
                    nc.gpsimd.dma_gather(
                        g[:, b0:b1, :], src[lo:hi, :],
                        ix[:, b0 * 8:b1 * 8], (b1 - b0) * P, (b1 - b0) * P,
                        cfg.F, queue_num=qctr % 4)
                    qctr += 1
            iofs += P * w8
            sl = spool.tile([P, nb, cfg.F], bf16, tag="S")
            q0 = 0
            for bi, b in enumerate(blocks):
                pt = ps[bi // 4][:, bi % 4, :]
                # one accumulation group per PSUM bank: start clears
                # has_written bank-wide, so only the first matmul into the
                # bank may set it; per-element has_written handles the
                # disjoint block slices.
                last_in_bank = bi % 4 == 3 or bi == len(blocks) - 1
                for q in range(NB[b, c]):
                    nc.vector.tensor_scalar(
                        sl[:, q0 + q, :], iota_t[:],
                        dvt[:, q0 + q, 0:1], dvt[:, q0 + q, 1:2],
                        op0=iseq, op1=mult)
                    nc.tensor.matmul(
                        pt, sl[:, q0 + q, :], g[:, q0 + q, :],
                        start=(c == 0 and q == 0 and bi % 4 == 0),
                        stop=(c == cfg.NCHUNK - 1 and q == NB[b, c] - 1
                              and last_in_bank),
                        skip_group_check=True)
                q0 += NB[b, c]
        for bi, b in enumerate(blocks):
            pt = ps[bi // 4][:, bi % 4, :]
            nrows = cfg.nrows_of(b)
            r0 = b * P
            ev = evpool.tile([P, cfg.F], f32, tag="ev")
            if prev is None:
                nc.vector.tensor_copy(ev[:nrows, :], pt[:nrows, :])
            else:
                pv = evpool.tile([P, cfg.F], f32, tag="pv")
                nc.sync.dma_start(pv[:nrows, :], prev[r0:r0 + nrows, :])
                nc.vector.tensor_scalar_mul(ev[:nrows, :], pt[:nrows, :], 2.0)
                nc.vector.tensor_tensor(ev[:nrows, :], ev[:nrows, :],
                                        pv[:nrows, :], op=sub)
            nc.scalar.dma_start(dst[r0:r0 + nrows, :], ev[:nrows, :])
            if bdst is not None:
                evb = evpool.tile([P, cfg.F], bf16, tag="evb")
                nc.vector.tensor_copy(evb[:nrows, :], ev[:nrows, :])
                nc.scalar.dma_start(bdst[r0:r0 + nrows, :], evb[:nrows, :])


def make_inputs(cfg, meta, per_core, x, W, b):
    import ml_dtypes
    x = np.asarray(x, dtype=np.float32)
    W = np.asarray(W, dtype=np.float32)
    b = np.asarray(b, dtype=np.float32)
    # w_lhsT[f, k, o] = W[o, f*K + k]
    wl = W.reshape(cfg.F, cfg.F, cfg.K).transpose(1, 2, 0)  # W[o, f, k] -> [f,k,o]
    wl = np.ascontiguousarray(wl).reshape(cfg.F, cfg.K * cfg.F)
    x_bf = x.astype(ml_dtypes.bfloat16)
    iota = np.tile(np.arange(P, dtype=np.float32).astype(ml_dtypes.bfloat16),
                   (P, 1))
    in_maps = []
    for cid in range(cfg.CORES):
        in_maps.append({
            "x_e": np.ascontiguousarray(x_bf[per_core[cid]["src_mat"]]),
            "x_shard": np.ascontiguousarray(
                x[cid * cfg.RPC:(cid + 1) * cfg.RPC]),
            "idx_all": per_core[cid]["idx_all"],
            "dv_all": per_core[cid]["dv_all"],
            "iota_rep": iota,
            "w_lhsT": wl,
            "b_row": b.reshape(1, cfg.F),
        })
    return in_maps


def kernel(x, lap_rows, lap_cols, lap_vals, W, b, k):
    cfg = Cfg()
    assert int(k) == cfg.K
    meta, per_core = preprocess(cfg, lap_rows, lap_cols, lap_vals)
    nc = build(cfg, meta)
    in_maps = make_inputs(cfg, meta, per_core, x, W, b)
    res = bass_utils.run_bass_kernel_spmd(
        nc, in_maps, core_ids=list(range(cfg.CORES)))
    out = np.concatenate([res.results[c]["out_shard"]
                          for c in range(cfg.CORES)], axis=0)
    return out.astype(np.float32)


# revision 6
# speedup vs baseline: 1.0663x; 1.0663x over previous
"""ChebNetConv (K=4) Bass kernel for 8 trn2 NeuronCores.

Strategy (1D row partitioning per sharding hint):
  - Nodes sharded across 8 cores (12500 rows each). Each SpMM step computes
    the core's own output rows; full neighbor tables (T1 / T2) are available
    to every core via AllGather.
  - SpMM core: edges grouped by (dest block of 128 rows, src chunk of 25000
    rows), padded to batches of 128.  Per batch: source rows land in an SBUF
    tile G[128e, 128f]; a selector tile S[128e, 128d] (Laplacian value at
    (e, dest-in-block)) is built ON-CHIP by the vector engine from a compact
    (dst, val) stream: S = (iota == dst) * val; PE matmul accumulates
    S.T @ G into the dest block's PSUM accumulator.
  - Step 1 needs no gather at all: x is a host input, so the edge-ordered
    gather x[cols] is precomputed on the host and streamed contiguously.
    Steps 2/3 use gpsimd dma_gather from the all-gathered T1/T2 tables.
  - Chebyshev recurrence (T2 = 2*L@T1 - T0) folded into PSUM eviction.
  - Final linear: per dest block, PE-transpose cheb tiles to [f, n] and
    accumulate 4 matmuls against W slices + bias outer product.
"""

import numpy as np

import concourse.bacc as bacc
import concourse.bass as bass
import concourse.mybir as mybir
import concourse.tile as tile
from concourse import bass_utils
from concourse.bass import ds
from concourse.masks import make_identity

P = 128


class Cfg:
    def __init__(self, n_nodes=100000, f=128, k=4, cores=8, chunk=25000, superb=8,
                 maxb=8):
        assert n_nodes % cores == 0
        self.N = n_nodes
        self.F = f
        self.K = k
        self.CORES = cores
        self.RPC = n_nodes // cores            # rows per core
        self.NBLK = -(-self.RPC // P)          # dest blocks per core
        self.CHUNK = chunk                     # gather-table chunk rows (int16 idx limit)
        assert chunk <= 32767
        self.NCHUNK = -(-n_nodes // chunk)
        self.SUPER = superb                    # dest blocks per super-block
        self.NSUPER = -(-self.NBLK // superb)
        self.MAXB = maxb                       # batches per dma_gather sub-call

    def blocks_of(self, s):
        return range(s * self.SUPER, min(self.NBLK, (s + 1) * self.SUPER))

    def nrows_of(self, b):
        return min(P, self.RPC - b * P)


def preprocess(cfg, rows, cols, vals):
    """Build per-core gather-index, (dst,val), and source-index streams.

    Returns (meta, per_core) where meta has compile-time batch counts
    (identical across cores) and per_core[c] = dict of arrays.
    """
    import ml_dtypes
    rows = np.asarray(rows).astype(np.int64)
    cols = np.asarray(cols).astype(np.int64)
    vals = np.asarray(vals).astype(np.float32)

    core = rows // cfg.RPC
    loc = rows % cfg.RPC
    blk = loc // P
    dst = loc % P
    chk = cols // cfg.CHUNK
    src = (cols % cfg.CHUNK).astype(np.int64)

    counts = np.zeros((cfg.CORES, cfg.NBLK, cfg.NCHUNK), dtype=np.int64)
    np.add.at(counts, (core, blk, chk), 1)
    NB = np.maximum(1, -(-counts.max(axis=0) // P))  # [NBLK, NCHUNK] batches

    slot_start = np.zeros((cfg.NBLK, cfg.NCHUNK), dtype=np.int64)
    call_start = {}          # (s, c) -> padded-edge offset of the gather call
    call_nbatch = {}         # (s, c) -> total batches in call
    off = 0
    for s in range(cfg.NSUPER):
        for c in range(cfg.NCHUNK):
            call_start[(s, c)] = off
            nb = 0
            for b in cfg.blocks_of(s):
                slot_start[b, c] = off
                off += NB[b, c] * P
                nb += NB[b, c]
            call_nbatch[(s, c)] = nb
    tot_pad = off
    NBT = tot_pad // P

    meta = dict(NB=NB, call_start=call_start, call_nbatch=call_nbatch,
                tot_pad=tot_pad, NBT=NBT)

    per_core = []
    for cid in range(cfg.CORES):
        m = core == cid
        key = (blk[m] * cfg.NCHUNK + chk[m])
        order = np.argsort(key, kind="stable")
        kb, kc, ksrc, kdst, kval = (blk[m][order], chk[m][order],
                                    src[m][order], dst[m][order],
                                    vals[m][order])
        # rank within slot
        cnt = counts[cid].reshape(-1)
        slot_flat = kb * cfg.NCHUNK + kc
        starts = np.zeros(cfg.NBLK * cfg.NCHUNK, dtype=np.int64)
        starts[1:] = np.cumsum(cnt)[:-1]
        rank = np.arange(len(kb)) - starts[slot_flat]
        pos = slot_start.reshape(-1)[slot_flat] + rank  # padded global position

        q = pos // P          # batch index
        e = pos % P           # lane within batch

        idx_flat = np.zeros(tot_pad, dtype=np.int16)
        idx_flat[pos] = ksrc.astype(np.int16)

        # compact per-lane (dst, val) stream [128, NBT, 2] bf16
        dv = np.zeros((P, NBT, 2), dtype=ml_dtypes.bfloat16)
        dv[e, q, 0] = kdst.astype(ml_dtypes.bfloat16)
        dv[e, q, 1] = kval.astype(ml_dtypes.bfloat16)

        # global source index per lane [128, NBT] (padding -> row 0, val=0)
        src_mat = np.zeros((P, NBT), dtype=np.int64)
        src_mat[e, q] = kc * cfg.CHUNK + ksrc

        # idx DMA layout: per call, [128, 8*nb] with idx j at
        # [16g + j%16, j//16] for replica groups g=0..7
        idx_parts = []
        for s in range(cfg.NSUPER):
            for c in range(cfg.NCHUNK):
                o = call_start[(s, c)]
                nb = call_nbatch[(s, c)]
                iv = idx_flat[o:o + nb * P]            # [nb*128]
                arr = iv.reshape(-1, 16).T             # [16, 8*nb]
                idx_parts.append(np.tile(arr, (8, 1)).reshape(-1))
        per_core.append(dict(
            idx_all=np.concatenate(idx_parts),
            dv_all=dv,
            src_mat=src_mat,
        ))
    return meta, per_core


def len_idx(cfg, meta):
    return meta["tot_pad"] * 8  # 128 parts * 8*nb cols per call of nb*128 idxs


def build(cfg, meta):
    """Build the Bass program. Returns nc."""
    NB = meta["NB"]
    NBT = meta["NBT"]
    f32 = mybir.dt.float32
    bf16 = mybir.dt.bfloat16
    nc = bacc.Bacc("TRN2", target_bir_lowering=False, debug=False,
                   num_devices=cfg.CORES, num_swdge_queues=4)

    x_e = nc.dram_tensor("x_e", [P, NBT, cfg.F], bf16, kind="ExternalInput")
    x_shard = nc.dram_tensor("x_shard", [cfg.RPC, cfg.F], f32,
                             kind="ExternalInput")
    idx_in = nc.dram_tensor("idx_all", [len_idx(cfg, meta)], mybir.dt.int16,
                            kind="ExternalInput")
    dv_in = nc.dram_tensor("dv_all", [P, NBT, 2], bf16, kind="ExternalInput")
    iota_in = nc.dram_tensor("iota_rep", [P, P], bf16, kind="ExternalInput")
    w_in = nc.dram_tensor("w_lhsT", [cfg.F, cfg.K * cfg.F], f32,
                          kind="ExternalInput")
    b_in = nc.dram_tensor("b_row", [1, cfg.F], f32, kind="ExternalInput")
    out_shard = nc.dram_tensor("out_shard", [cfg.RPC, cfg.F], f32,
                               kind="ExternalOutput")

    rg = [list(range(cfg.CORES))]

    with tile.TileContext(nc) as tc:
        with tc.tile_pool(name="dram", bufs=1, space="DRAM") as dram:
            t1_shard = dram.tile([cfg.RPC, cfg.F], f32, tag="t1s")
            t2_shard = dram.tile([cfg.RPC, cfg.F], f32, tag="t2s")
            t3_shard = dram.tile([cfg.RPC, cfg.F], f32, tag="t3s")
            t1_tab = dram.tile([cfg.N, cfg.F], bf16, tag="t1t",
                               addr_space="Shared")
            t2_tab = dram.tile([cfg.N, cfg.F], bf16, tag="t2t",
                               addr_space="Shared")
            t1_bsh = dram.tile([cfg.RPC, cfg.F], bf16, tag="t1b")
            t2_bsh = dram.tile([cfg.RPC, cfg.F], bf16, tag="t2b")

            with (
                tc.tile_pool(name="gconst", bufs=1) as gconst,
                tc.tile_pool(name="gpool", bufs=2) as gpool,
                tc.tile_pool(name="spool", bufs=2) as spool,
                tc.tile_pool(name="ipool", bufs=2) as ipool,
                tc.tile_pool(name="dvpool", bufs=2) as dvpool,
                tc.tile_pool(name="psum", bufs=2, space="PSUM") as pspool,
                tc.tile_pool(name="ev", bufs=4) as evpool,
            ):
                iota_t = gconst.tile([P, P], bf16)
                nc.sync.dma_start(iota_t[:], iota_in[:])
                for step in (1, 2, 3):
                    src = {1: None, 2: t1_tab[:], 3: t2_tab[:]}[step]
                    prev = {1: None, 2: x_shard, 3: t1_shard}[step]
                    dst = {1: t1_shard, 2: t2_shard, 3: t3_shard}[step]
                    bdst = {1: t1_bsh, 2: t2_bsh, 3: None}[step]
                    spmm_step(cfg, meta, nc, tc, gpool, spool, ipool, dvpool,
                              pspool, evpool, idx_in, dv_in, iota_t, x_e, src,
                              prev, dst, step, bdst)
                    if step == 1:
                        nc.gpsimd.collective_compute(
                            "AllGather", mybir.AluOpType.bypass,
                            replica_groups=rg, ins=[t1_bsh[:].opt()],
                            outs=[t1_tab[:].opt()])
                    elif step == 2:
                        nc.gpsimd.collective_compute(
                            "AllGather", mybir.AluOpType.bypass,
                            replica_groups=rg, ins=[t2_bsh[:].opt()],
                            outs=[t2_tab[:].opt()])

            with (
                tc.tile_pool(name="fconst", bufs=1) as fconst,
                tc.tile_pool(name="fload", bufs=3) as fload,
                tc.tile_pool(name="ftrans", bufs=3) as ftrans,
                tc.tile_pool(name="fpsum", bufs=2, space="PSUM") as fpsum,
                tc.tile_pool(name="fout", bufs=3) as foutp,
            ):
                ident = fconst.tile([P, P], f32)
                make_identity(nc, ident[:])
                wt = fconst.tile([cfg.F, cfg.K, cfg.F], f32)
                nc.sync.dma_start(wt[:], w_in[:].rearrange(
                    "f (k o) -> f k o", k=cfg.K))
                brow = fconst.tile([1, cfg.F], f32)
                nc.sync.dma_start(brow[:], b_in[:])
                ones = fconst.tile([1, P], f32)
                nc.vector.memset(ones[:], 1.0)

                shards = [x_shard, t1_shard, t2_shard, t3_shard]
                for b in range(cfg.NBLK):
                    nrows = cfg.nrows_of(b)
                    r0 = b * P
                    opsum = fpsum.tile([P, cfg.F], f32, tag="opsum")
                    for k in range(cfg.K):
                        ct = fload.tile([P, cfg.F], f32, tag="cheb")
                        sh = shards[k]
                        nc.sync.dma_start(ct[:nrows, :],
                                          sh[r0:r0 + nrows, :])
                        tp = fpsum.tile([P, P], f32, tag="tpsum")
                        nc.tensor.transpose(tp[:, :nrows], ct[:nrows, :],
                                            ident[:nrows, :nrows])
                        cT = ftrans.tile([cfg.F, P], f32, tag="chebT")
                        nc.vector.tensor_copy(cT[:, :nrows], tp[:, :nrows])
                        nc.tensor.matmul(opsum[:nrows, :], cT[:, :nrows],
                                         wt[:, k, :], start=(k == 0),
                                         stop=False)
                    nc.tensor.matmul(opsum[:nrows, :], ones[:1, :nrows],
                                     brow[:1, :], start=False, stop=True)
                    ot = foutp.tile([P, cfg.F], f32, tag="ot")
                    nc.vector.tensor_copy(ot[:nrows, :], opsum[:nrows, :])
                    nc.scalar.dma_start(out_shard[r0:r0 + nrows, :],
                                        ot[:nrows, :])

    nc.compile()
    return nc


def spmm_step(cfg, meta, nc, tc, gpool, spool, ipool, dvpool, pspool, evpool,
              idx_in, dv_in, iota_t, x_e, src, prev, dst, step, bdst=None):
    NB = meta["NB"]
    f32 = mybir.dt.float32
    bf16 = mybir.dt.bfloat16
    sub = mybir.AluOpType.subtract
    iseq = mybir.AluOpType.is_equal
    mult = mybir.AluOpType.mult
    iofs = 0
    qctr = 0
    for s in range(cfg.NSUPER):
        blocks = list(cfg.blocks_of(s))
        ps = [pspool.tile([P, 4, cfg.F], f32, tag=f"ps{i}", name=f"ps{i}")
              for i in range(-(-len(blocks) // 4))]
        for c in range(cfg.NCHUNK):
            nb = meta["call_nbatch"][(s, c)]
            bofs = meta["call_start"][(s, c)] // P   # batch offset in streams
            w8 = nb * 8
            dvt = dvpool.tile([P, nb, 2], bf16, tag="dv")
            nc.sync.dma_start(dvt[:], dv_in[:, bofs:bofs + nb, :])
            g = gpool.tile([P, nb, cfg.F], bf16, tag="G")
            if step == 1:
                nc.sync.dma_start(g[:], x_e[:, bofs:bofs + nb, :])
            else:
                ix = ipool.tile([P, w8], mybir.dt.int16, tag="ix")
                nc.sync.dma_start(
                    ix[:], idx_in[iofs:iofs + P * w8].rearrange(
                        "(p w) -> p w", p=P))
                lo = c * cfg.CHUNK
                hi = min(cfg.N, lo + cfg.CHUNK)
                # split into sub-calls: very large dma_gather calls (~10k
                # descriptors) crash/hang the device
                for b0 in range(0, nb, cfg.MAXB):
                    b1 = min(nb, b0 + cfg.MAXB)
                    nc.gpsimd.dma_gather(
                        g[:, b0:b1, :], src[lo:hi, :],
                        ix[:, b0 * 8:b1 * 8], (b1 - b0) * P, (b1 - b0) * P,
                        cfg.F, queue_num=qctr % 4)
                    qctr += 1
            iofs += P * w8
            sl = spool.tile([P, nb, cfg.F], bf16, tag="S")
            # whole-call selector build: S[e,q,d] = (iota[d]==dst[e,q])*val[e,q]
            iv = iota_t[:]
            iota_b = bass.AP(iv.tensor, iv.offset,
                             [iv.ap[0], [0, nb], iv.ap[1]])
            dstv = dvt[:, :, 0:1]
            dst_b = bass.AP(dstv.tensor, dstv.offset,
                            [dstv.ap[0], dstv.ap[1], [0, cfg.F]])
            valv = dvt[:, :, 1:2]
            val_b = bass.AP(valv.tensor, valv.offset,
                            [valv.ap[0], valv.ap[1], [0, cfg.F]])
            nc.vector.tensor_tensor(sl[:], iota_b, dst_b, op=iseq)
            nc.vector.tensor_tensor(sl[:], sl[:], val_b, op=mult)
            q0 = 0
            for bi, b in enumerate(blocks):
                pt = ps[bi // 4][:, bi % 4, :]
                # one accumulation group per PSUM bank: start clears
                # has_written bank-wide, so only the first matmul into the
                # bank may set it; per-element has_written handles the
                # disjoint block slices.
                last_in_bank = bi % 4 == 3 or bi == len(blocks) - 1
                for q in range(NB[b, c]):
                    nc.tensor.matmul(
                        pt, sl[:, q0 + q, :], g[:, q0 + q, :],
                        start=(c == 0 and q == 0 and bi % 4 == 0),
                        stop=(c == cfg.NCHUNK - 1 and q == NB[b, c] - 1
                              and last_in_bank),
                        skip_group_check=True)
                q0 += NB[b, c]
        for bi, b in enumerate(blocks):
            pt = ps[bi // 4][:, bi % 4, :]
            nrows = cfg.nrows_of(b)
            r0 = b * P
            ev = evpool.tile([P, cfg.F], f32, tag="ev")
            if prev is None:
                nc.vector.tensor_copy(ev[:nrows, :], pt[:nrows, :])
            else:
                pv = evpool.tile([P, cfg.F], f32, tag="pv")
                nc.sync.dma_start(pv[:nrows, :], prev[r0:r0 + nrows, :])
                nc.vector.tensor_scalar_mul(ev[:nrows, :], pt[:nrows, :], 2.0)
                nc.vector.tensor_tensor(ev[:nrows, :], ev[:nrows, :],
                                        pv[:nrows, :], op=sub)
            nc.scalar.dma_start(dst[r0:r0 + nrows, :], ev[:nrows, :])
            if bdst is not None:
                evb = evpool.tile([P, cfg.F], bf16, tag="evb")
                nc.vector.tensor_copy(evb[:nrows, :], ev[:nrows, :])
                nc.scalar.dma_start(bdst[r0:r0 + nrows, :], evb[:nrows, :])


def make_inputs(cfg, meta, per_core, x, W, b):
    import ml_dtypes
    x = np.asarray(x, dtype=np.float32)
    W = np.asarray(W, dtype=np.float32)
    b = np.asarray(b, dtype=np.float32)
    # w_lhsT[f, k, o] = W[o, f*K + k]
    wl = W.reshape(cfg.F, cfg.F, cfg.K).transpose(1, 2, 0)  # W[o, f, k] -> [f,k,o]
    wl = np.ascontiguousarray(wl).reshape(cfg.F, cfg.K * cfg.F)
    x_bf = x.astype(ml_dtypes.bfloat16)
    iota = np.tile(np.arange(P, dtype=np.float32).astype(ml_dtypes.bfloat16),
                   (P, 1))
    in_maps = []
    for cid in range(cfg.CORES):
        in_maps.append({
            "x_e": np.ascontiguousarray(x_bf[per_core[cid]["src_mat"]]),
            "x_shard": np.ascontiguousarray(
                x[cid * cfg.RPC:(cid + 1) * cfg.RPC]),
            "idx_all": per_core[cid]["idx_all"],
            "dv_all": per_core[cid]["dv_all"],
            "iota_rep": iota,
            "w_lhsT": wl,
            "b_row": b.reshape(1, cfg.F),
        })
    return in_maps


def kernel(x, lap_rows, lap_cols, lap_vals, W, b, k):
    cfg = Cfg()
    assert int(k) == cfg.K
    meta, per_core = preprocess(cfg, lap_rows, lap_cols, lap_vals)
    nc = build(cfg, meta)
    in_maps = make_inputs(cfg, meta, per_core, x, W, b)
    res = bass_utils.run_bass_kernel_spmd(
        nc, in_maps, core_ids=list(range(cfg.CORES)))
    out = np.concatenate([res.results[c]["out_shard"]
                          for c in range(cfg.CORES)], axis=0)
    return out.astype(np.float32)
